# revision 3
# baseline (speedup 1.0000x reference)
"""ExpandedPerformerFeatureMap TRN2 Bass kernel, v4.

out[r, m] = exp(proj[r, m] + c[r]),  proj = xs @ W^T,  xs = x * d^-0.25,
c[r] = -0.5*||xs_r||^2 - ln(16)

v3 vs v2: every matmul has K >= 64 and K=128 matmuls recur, because the
PE activity monitor (HAM) only unthrottles 1.2 -> 2.4 GHz for full-K
work: K=1 rank-1 bias matmuls actively re-throttle it (measured).
 - warm-pump: ~20 K=128 matmuls at kernel start trigger 2.4 GHz while
   the first x chunk loads.
 - bias add is a K=128 matmul: lhsT = c-tile (rows 0/1 = c_even/c_odd,
   rows 2-127 zeroed once per buffer), rhs = parity mask [128, 512].
 - per tile one PSUM accumulation group: projE(start) -> projO -> bias.
Everything else as v2: fp16 host-packed paired-transpose input, fp16
output (host upcast), sumsq via sel matmul, bias-free big-span ACT Exp.
"""

import numpy as np

import concourse.bass as bass
import concourse.tile as tile
from concourse import mybir
from concourse.bass import compact_to_ranges
from concourse.bass_utils import run_bass_kernel_spmd

B, H, L, D = 4, 16, 4096, 64
M = 256
N_CORES = 8
ROWS = B * H * L
ROWS_PER_CORE = ROWS // N_CORES    # 32768
PAIRS = ROWS_PER_CORE // 2         # 16384
NT = PAIRS // 128                  # 128 tiles of 128 pairs
LCHUNK = 2048                      # pairs per input DMA
PCH = 3                            # tiles per PSUM/ACT chunk (3 banks)
CGRP = 4                           # tiles per sumsq matmul group
CSB_BUFS = 6
N_PUMP = 14

SCALE = float(D) ** -0.25
NEG_LN16 = -float(np.log(M)) / 2.0

FP32 = mybir.dt.float32
F16 = mybir.dt.float16


# --- walrus build workarounds (see kernel2) ---------------------------------

def _clear_and_free_semaphores_no_rangeclear(self, sems):
    if not sems:
        return
    sem_nums = [s.num if hasattr(s, "num") else s for s in sems]
    for sem_range in compact_to_ranges(sem_nums):
        assert self._state.free_isdisjoint(sem_range)
        self.gpsimd.dma_reset(sem_range)
    self._state.prepend_free_semaphores(sem_nums)
    for poison_set in self._tile_sem_poison_stack:
        poison_set.update(sem_nums)


def _drain_and_barrier_trim(self, tick_clock, wait_clock):
    from concourse.vector_clock import ScopedClock

    drain_inst = self.nc.sync.drain()
    wait_clock.add_sem_waits(
        drain_inst.ins, ScopedClock({None: tick_clock.global_clock})
    )
    self.nc.all_engine_barrier()
    popped = self.nc._tile_sem_poison_stack.pop()
    assert popped is self._sem_poison
    sems = list(self.sems.allocated().values())
    sem_nums = [s.num if hasattr(s, "num") else s for s in sems]
    self.nc._state.prepend_free_semaphores(sem_nums)
    for poison_set in self.nc._tile_sem_poison_stack:
        poison_set.update(sem_nums)


def _split_excess_waits(nc):
    n_new = 0
    for func in nc.m.functions:
        for block in func.blocks:
            new_insts = []
            for inst in block.instructions:
                si = getattr(inst, "sync_info", None)
                waits = list(si.on_wait) if si is not None and si.on_wait else []
                if len(waits) > 1:
                    for w in waits[:-1]:
                        n_new += 1
                        nop = mybir.InstNoOp(
                            name=f"{inst.name}-xw{n_new}", ins=[], outs=[]
                        )
                        nop.engine = inst.engine
                        nop.sync_info = mybir.SyncInfo(on_wait=[w], on_update=[])
                        new_insts.append(nop)
                    si.on_wait = [waits[-1]]
                new_insts.append(inst)
            if n_new:
                block.instructions[:] = new_insts
    return n_new


def _build_kernel(nc: bass.Bass):
    x_ap = nc.dram_tensor("x2t", [128, PAIRS], F16, kind="ExternalInput").ap()
    cst_ap = nc.dram_tensor("cst", [128, 2 * M + 2], F16, kind="ExternalInput").ap()
    # out layout [partition, tile, (parity, m)]: each partition's stores are
    # contiguous in HBM (strided 1KB-chunk stores measured ~270 GB/s vs ~350
    # contiguous); host restores row order with one transpose.
    out_ap = nc.dram_tensor(
        "out", [128, NT, 2 * M], F16, kind="ExternalOutput"
    ).ap()

    n_chunks = (NT + PCH - 1) // PCH

    with tile.TileContext(nc) as tc:
        with (
            tc.tile_pool(name="consts", bufs=1) as consts,
            tc.tile_pool(name="xin", bufs=4) as xin_pool,
            tc.tile_pool(name="xsq", bufs=3) as xsq_pool,
            tc.tile_pool(name="outp", bufs=4) as out_pool,
            tc.tile_pool(name="expc", bufs=3) as expc_pool,
            tc.tile_pool(name="psproj", bufs=2, space="PSUM") as ps_proj,
            tc.tile_pool(name="psc", bufs=2, space="PSUM") as ps_c,
        ):
            # pump source available at t~0 (no DMA dependency)
            pump_src = consts.tile([128, M], F16)
            nc.vector.memset(pump_src[:], 0.125)
            ln16_sb = consts.tile([128, 1], FP32)
            nc.vector.memset(ln16_sb[:], NEG_LN16)

            # --- one-time constants, one DMA: w | mask2 | sel ---
            cst_sb = consts.tile([128, 2 * M + 2], F16)
            nc.sync.dma_start(out=cst_sb[:], in_=cst_ap)
            w_sb = cst_sb[:, 0 : 2 * M]
            selp_sb = cst_sb[:, 2 * M : 2 * M + 2]

            # warm the ACT exp table (runs off pump_src, no DMA wait)
            warm = consts.tile([1, M], F16)
            nc.scalar.activation(
                out=warm[:], in_=pump_src[0:1, :],
                func=mybir.ActivationFunctionType.Exp,
            )

            # Prefetch the first two x chunks (and their squares/sumsq
            # inputs) before the warm-pump so the pump covers their latency
            # and the PE never idles at pipeline start.
            pre_loads = []
            for lc0 in range(2):
                xt0 = xin_pool.tile([128, LCHUNK], F16, tag="xt", name=f"xt_pre{lc0}")
                nc.sync.dma_start(
                    out=xt0[:], in_=x_ap[:, lc0 * LCHUNK : (lc0 + 1) * LCHUNK]
                )
                xq0 = xsq_pool.tile([128, LCHUNK], F16, tag="xq", name=f"xq_pre{lc0}")
                nc.vector.tensor_mul(xq0[:], xt0[:], xt0[:])
                pre_loads.append((xt0, xq0))

            # HAM warm-pump: K=128 matmuls push the PE to 2.4 GHz while the
            # first x chunks load. Spread across banks to avoid WAW waits.
            wp0 = ps_proj.tile([128, PCH, 2, M], FP32, tag="proj")
            wp1 = ps_proj.tile([128, PCH, 2, M], FP32, tag="proj")
            for wi in range(N_PUMP):
                wt = (wp0, wp1)[(wi // PCH) % 2]
                nc.tensor.matmul(
                    wt[:, wi % PCH, (wi // (2 * PCH)) % 2, :],
                    pump_src[:, 0:128], pump_src[:],
                    start=True, stop=True,
                )

            TPL = LCHUNK // 128
            GPL = TPL // CGRP
            xts = [None] * (NT // TPL)
            xqs = [None] * (NT // TPL)
            pscs = [None] * (NT // TPL)
            cs = [None] * (NT // TPL)
            n_c_alloc = 0

            cur_osb = None
            osb_base = 0

            for pc in range(n_chunks):
                t0 = pc * PCH
                nt = min(PCH, NT - t0)
                ps = ps_proj.tile([128, PCH, 2, M], FP32, tag="proj")

                for j in range(nt):
                    t = t0 + j
                    lc, ti = divmod(t, TPL)

                    # prefetch loads two chunks ahead so squares/sumsq
                    # inputs are ready long before first use
                    if ti == 0:
                        for lcp in (lc, lc + 2):
                            if lcp < len(xts) and xqs[lcp] is None:
                                if lcp < 2:
                                    xtp, xqp = pre_loads[lcp]
                                else:
                                    xtp = xin_pool.tile(
                                        [128, LCHUNK], F16, tag="xt", name=f"xt{lcp}"
                                    )
                                    nc.sync.dma_start(
                                        out=xtp[:],
                                        in_=x_ap[:, lcp * LCHUNK : (lcp + 1) * LCHUNK],
                                    )
                                    xqp = xsq_pool.tile(
                                        [128, LCHUNK], F16, tag="xq", name=f"xq{lcp}"
                                    )
                                    nc.vector.tensor_mul(xqp[:], xtp[:], xtp[:])
                                xts[lcp], xqs[lcp] = xtp, xqp

                    # sumsq matmuls in two staggered half-bursts (a single
                    # 16-matmul burst stalls proj production past the ACT's
                    # one-chunk PSUM lead and starves the Exp stream)
                    if ti == 0 or ti == 4:
                        half = 0 if ti == 0 else 1
                        xq = xqs[lc]
                        if half == 0:
                            psc = ps_c.tile([128, TPL, 2], FP32, tag="psc")
                            expc = expc_pool.tile([128, TPL, 2], FP32, tag="expc")
                            pscs[lc], cs[lc] = psc, expc
                        psc, expc = pscs[lc], cs[lc]
                        u0, u1 = (0, 8) if half == 0 else (8, TPL)
                        for u in range(u0, u1):
                            nc.tensor.matmul(
                                psc[:, u, :],
                                xq[:, u * 128 : (u + 1) * 128],
                                selp_sb[:],
                                start=True, stop=True,
                            )
                        nc.scalar.activation(
                            out=expc[:, u0:u1, :], in_=psc[:, u0:u1, :],
                            func=mybir.ActivationFunctionType.Exp,
                            bias=ln16_sb[:], scale=-0.5,
                        )

                    xt = xts[lc]

                    # One K=128 N=512 proj matmul per tile (block-diagonal
                    # W covers both parities). All-K=128 keeps the PE
                    # activity monitor at 2.4 GHz.
                    nc.tensor.matmul(
                        ps[:, j, :, :],
                        xt[:, ti * 128 : (ti + 1) * 128],
                        w_sb[:],
                        start=True, stop=True,
                    )

                if pc % 2 == 0:
                    cur_osb = out_pool.tile([128, 2, PCH, 2 * M], F16, tag="osb")
                    osb_base = t0
                    osb_stored = 0

                nc.scalar.activation(
                    out=cur_osb[:, pc % 2, 0:nt, :],
                    in_=ps[:, 0:nt, :, :],
                    func=mybir.ActivationFunctionType.Exp,
                )
                # multiply exp(proj) by exp(c): one tensor_scalar per
                # (tile, parity) slice with a per-partition f32 scalar AP
                for j in range(nt):
                    lc0, u0 = divmod(t0 + j, TPL)
                    for q in range(2):
                        sl = cur_osb[:, pc % 2, j, q * M : (q + 1) * M]
                        nc.vector.tensor_scalar(
                            out=sl, in0=sl,
                            scalar1=cs[lc0][:, u0, q : q + 1], scalar2=None,
                            op0=mybir.AluOpType.mult,
                        )

                last = pc == n_chunks - 1
                if pc % 2 == 1 or last or pc >= n_chunks - 3:
                    ntiles = t0 + nt - osb_base
                    nc.sync.dma_start(
                        out=out_ap[:, osb_base + osb_stored : osb_base + ntiles, :],
                        in_=cur_osb[:].rearrange("p a b c -> p (a b) c")[
                            :, osb_stored:ntiles, :
                        ],
                    )
                    osb_stored = ntiles

    return nc


_NC_CACHE = None


def _get_nc():
    global _NC_CACHE
    if _NC_CACHE is None:
        orig = bass.Bass.clear_and_free_semaphores
        orig_dab = tile.TileContext._drain_and_barrier
        bass.Bass.clear_and_free_semaphores = _clear_and_free_semaphores_no_rangeclear
        tile.TileContext._drain_and_barrier = _drain_and_barrier_trim
        try:
            nc = bass.Bass("TRN2", target_bir_lowering=False, debug=False,
                           num_devices=N_CORES)
            _build_kernel(nc)
        finally:
            bass.Bass.clear_and_free_semaphores = orig
            tile.TileContext._drain_and_barrier = orig_dab
        _split_excess_waits(nc)
        _NC_CACHE = nc
    return _NC_CACHE


def kernel(x: np.ndarray, random_feats: np.ndarray, _trace=False, _tmpdir=None):
    nc = _get_nc()
    xs = np.asarray(x, dtype=np.float32).reshape(ROWS, D) * np.float32(SCALE)
    xs16 = xs.astype(np.float16)
    w16 = np.asarray(random_feats, dtype=np.float32).T.astype(np.float16)  # (64, 256)
    cst = np.zeros((128, 2 * M + 2), dtype=np.float16)
    cst[0:64, 0:M] = w16                      # w block-diag
    cst[64:128, M : 2 * M] = w16
    cst[0:64, 2 * M] = 1.0                    # parity select (sumsq)
    cst[64:128, 2 * M + 1] = 1.0

    in_maps = []
    for i in range(N_CORES):
        shard = xs16[i * ROWS_PER_CORE : (i + 1) * ROWS_PER_CORE]
        x2t = np.ascontiguousarray(
            shard.reshape(PAIRS, 2, D).transpose(1, 2, 0).reshape(128, PAIRS)
        )
        in_maps.append({"x2t": x2t, "cst": cst})

    res = run_bass_kernel_spmd(
        nc, in_maps, core_ids=list(range(N_CORES)), trace=_trace, tmpdir=_tmpdir
    )
    out = np.empty((ROWS, M), dtype=np.float32)
    for i in range(N_CORES):
        # [128 part, NT, 512] -> pair p = t*128 + i -> natural rows
        oc = res.results[i]["out"].reshape(128, NT, 2 * M).transpose(1, 0, 2)
        out[i * ROWS_PER_CORE : (i + 1) * ROWS_PER_CORE] = (
            np.ascontiguousarray(oc).reshape(ROWS_PER_CORE, M).astype(np.float32)
        )
    full = out.reshape(B, H, L, M)
    if _trace:
        return full, res
    return full
